# revision 18
# baseline (speedup 1.0000x reference)
"""Trainium2 Bass kernel for nn_CLinear_6768868459230.

Context-conditioned block-autoregressive linear layer (MAF-style):
  wdir = c @ Wd + bd                      [B, O, I]
  w    = exp(wdir)*mask_diag + wdir*mask_lower
  sqn  = sum(w^2, axis=I)
  y    = (w / sqrt(sqn) * exp(wamp)) @ xv + bias
  logdet = logsumexp over diag block of (wdir - 0.5 log sqn + wamp + xl)

Sharding: tensor-parallel over the O=512 output rows (the 262144-wide Wd
matmul dominates). Each of the 8 cores owns 8 of the 64 channels, chosen
as pairs {k, 15-k, 16+k, 31-k, ...} so the block-triangular work (rows of
channel ch touch only ch*8 input columns) is identical on every core —
required anyway because all cores execute one shared program.

Only the strictly-lower + diagonal columns of Wd are shipped/loaded
(the rest are masked to zero by the reference), roughly halving traffic.
Per-row lower widths are zero-padded up to a per-slot maximum W(j)=64j+56
so the instruction stream is core-independent; zero padding is exact
(contributes 0 to both sums).

On-device per core (per 128-sample batch chunk):
  TensorE : wdir lower segments + diag block via cT.T @ Wd (float32r),
            with bd added as K=1 ones-outer-product accumulating matmuls
  ScalarE : per-row sum(t^2) via activation(Square, accum_out)
  VectorE : per-row sum(t * xv) via tensor_tensor_reduce
  diag    : batched 512-wide: exp(td), exp(2 td), products with xv and
            exp(xl), segmented tensor_reduce over fin=8
  logdet  = wamp - 0.5*ln(sqn) + ln(sum_f exp(td + xl))  (no max-trick
            needed: |td + xl| <~ 8 at these scales)
"""

import numpy as np

NCH, FIN, FOUT, CDIM, B = 64, 8, 8, 128, 256
I = NCH * FIN
O = NCH * FOUT
NCORES = 8
NLOC = 64  # output rows per core
BCH = 128  # batch chunk (SBUF partitions)

# per-slot padded lower width and rows-per-matmul grouping
W_OF_J = [64 * j + 56 for j in range(8)]
G_OF_J = [8, 4, 2, 2, 1, 1, 1, 1]  # rows per matmul so N = G*W <= 512
BASE_J = [8 * sum(W_OF_J[:j]) for j in range(8)]
TOTW = 8 * sum(W_OF_J)  # 17920


def _channels(k):
    return [k, 15 - k, 16 + k, 31 - k, 32 + k, 47 - k, 48 + k, 63 - k]


_NC = None
_REPEAT = 1  # bench knob: replicate compute body


def _build_nc():
    import concourse.bacc as bacc
    import concourse.tile as tile
    from concourse import mybir

    f32 = mybir.dt.float32
    f32r = mybir.dt.float32r
    bf16 = mybir.dt.bfloat16
    AF = mybir.ActivationFunctionType
    ALU = mybir.AluOpType

    nc = bacc.Bacc(None, target_bir_lowering=False)

    d_cT = nc.dram_tensor("ct", [CDIM, B], bf16, kind="ExternalInput")
    d_wdl = nc.dram_tensor("wdl", [CDIM, TOTW], bf16, kind="ExternalInput")
    d_wdd = nc.dram_tensor("wdd", [CDIM, NLOC * FIN], bf16, kind="ExternalInput")
    d_bdd = nc.dram_tensor("bdd", [1, NLOC * FIN], bf16, kind="ExternalInput")
    d_ew = nc.dram_tensor("ew", [CDIM, 3 * NLOC], bf16, kind="ExternalInput")
    d_eb = nc.dram_tensor("eb", [1, 3 * NLOC], bf16, kind="ExternalInput")
    d_xvT = nc.dram_tensor("xvt", [I, B], bf16, kind="ExternalInput")
    d_bdm = nc.dram_tensor("bdm", [I, NLOC], bf16, kind="ExternalInput")
    d_xv = nc.dram_tensor("xv", [B, I], f32, kind="ExternalInput")
    d_xvd = nc.dram_tensor("xvd", [B, NLOC * FIN], f32, kind="ExternalInput")
    d_xle = nc.dram_tensor("xle", [B, NLOC * FIN], f32, kind="ExternalInput")
    d_out = nc.dram_tensor("out", [B, NLOC, 2], f32, kind="ExternalOutput")

    with tile.TileContext(nc) as tc:
        with (
            tc.tile_pool(name="consts", bufs=1) as consts,
            tc.tile_pool(name="scr", bufs=3) as scr,
            tc.tile_pool(name="accs", bufs=2) as accs,
            tc.tile_pool(name="segp", bufs=6, space="PSUM") as segp,
            tc.tile_pool(name="miscp", bufs=1, space="PSUM") as miscp,
            tc.tile_pool(name="extp", bufs=1, space="PSUM") as extp,
        ):
            # ---- constants / weights ----
            # weight DMAs first: big slots alternate between the two HWDGE
            # queues (sync + scalar) so they stream in parallel from t=0
            wdl_sb = []
            for j in range(8):
                w = W_OF_J[j]
                t = consts.tile([CDIM, 8 * w], bf16, name=f"wdl{j}", tag=f"wdl{j}")
                eng = nc.sync if j % 2 == 0 else nc.scalar
                eng.dma_start(out=t, in_=d_wdl[:, BASE_J[j] : BASE_J[j] + 8 * w])
                wdl_sb.append(t)
            ct_sb = consts.tile([CDIM, B], bf16)
            nc.sync.dma_start(out=ct_sb, in_=d_cT[:, :])
            ones_sb = consts.tile([1, BCH], bf16)
            nc.vector.memset(ones_sb, 1.0)
            wdd_sb = consts.tile([CDIM, NLOC * FIN], bf16)
            nc.sync.dma_start(out=wdd_sb, in_=d_wdd[:, :])
            bdd_sb = consts.tile([1, NLOC * FIN], bf16)
            nc.sync.dma_start(out=bdd_sb, in_=d_bdd[:, :])
            ew_sb = consts.tile([CDIM, 3 * NLOC], bf16)
            nc.sync.dma_start(out=ew_sb, in_=d_ew[:, :])
            eb_sb = consts.tile([1, 3 * NLOC], bf16)
            nc.sync.dma_start(out=eb_sb, in_=d_eb[:, :])
            xvt_sb = consts.tile([CDIM, 4, B], bf16)
            for kc in range(4):
                nc.sync.dma_start(
                    out=xvt_sb[:, kc, :], in_=d_xvT[kc * 128 : (kc + 1) * 128, :]
                )
            bdm_sb = consts.tile([CDIM, 4, NLOC], bf16)
            for kc in range(4):
                nc.sync.dma_start(
                    out=bdm_sb[:, kc, :], in_=d_bdm[kc * 128 : (kc + 1) * 128, :]
                )

            xv_sb, xvd_sb, xle_sb = [], [], []
            for bc in range(2):
                b0 = bc * BCH
                t = consts.tile([BCH, I], f32, name=f"xv{bc}", tag=f"xv{bc}")
                nc.sync.dma_start(out=t, in_=d_xv[b0 : b0 + BCH, :])
                xv_sb.append(t)
                t = consts.tile([BCH, NLOC * FIN], f32, name=f"xvd{bc}", tag=f"xvd{bc}")
                nc.sync.dma_start(out=t, in_=d_xvd[b0 : b0 + BCH, :])
                xvd_sb.append(t)
                t = consts.tile([BCH, NLOC * FIN], f32, name=f"xle{bc}", tag=f"xle{bc}")
                nc.sync.dma_start(out=t, in_=d_xle[b0 : b0 + BCH, :])
                xle_sb.append(t)


            for _rep in range(_REPEAT):
                BATCH_SQ = (0, 1)  # slots whose squares are segment-batched
                st = {}
                # ---- phase A: matmuls + squares + dot products ----
                for bc in range(2):
                    b0 = bc * BCH
                    lhs = ct_sb[:, b0 : b0 + BCH]
                    xv_b = xv_sb[bc]

                    # extras: wamp | bias | 2*c@g | dotbd
                    pex = extp.tile([BCH, 4 * NLOC], f32, name="pex", tag="pex")
                    nc.tensor.matmul(
                        pex[:, : 3 * NLOC], lhs, ew_sb, start=True, stop=False
                    )
                    nc.tensor.matmul(
                        pex[:, : 3 * NLOC], ones_sb, eb_sb, start=False, stop=True
                    )
                    for kc in range(4):
                        nc.tensor.matmul(
                            pex[:, 3 * NLOC :],
                            xvt_sb[:, kc, b0 : b0 + BCH],
                            bdm_sb[:, kc, :],
                            start=(kc == 0),
                            stop=(kc == 3),
                        )

                    # diag block matmul (elementwise work deferred to phase B)
                    pdg = miscp.tile([BCH, NLOC * FIN], f32, name="pdg", tag="pdg")
                    nc.tensor.matmul(pdg, lhs, wdd_sb, start=True, stop=False)
                    nc.tensor.matmul(pdg, ones_sb, bdd_sb, start=False, stop=True)

                    SQL = accs.tile([BCH, NLOC], f32, name="SQL", tag="SQL")
                    DOTL = accs.tile([BCH, NLOC], f32, name="DOTL", tag="DOTL")
                    for j in range(8):
                        w, g = W_OF_J[j], G_OF_J[j]
                        prodj = scr.tile(
                            [BCH, 8 * 504], f32, name="prodj", tag="prodj", bufs=2
                        )
                        sqbj = None
                        if j in BATCH_SQ:
                            sqbj = scr.tile(
                                [BCH, 8 * 120], f32, name="sqbj", tag="sqbj", bufs=2
                            )
                        for s in range(8 // g):
                            r0 = j * 8 + s * g
                            n = g * w
                            ps = segp.tile([BCH, 512], f32, name="ps", tag="ps")
                            nc.tensor.matmul(
                                ps[:, :n],
                                lhs,
                                wdl_sb[j][:, s * n : (s + 1) * n],
                                start=True,
                                stop=True,
                            )
                            if j in BATCH_SQ:
                                nc.scalar.activation(
                                    out=sqbj[:, s * n : (s + 1) * n],
                                    in_=ps[:, :n],
                                    func=AF.Square,
                                )
                            else:
                                for q in range(g):
                                    r = r0 + q
                                    a = q * w
                                    sS = scr.tile(
                                        [BCH, 504], f32, name="sS", tag="sS"
                                    )
                                    nc.scalar.activation(
                                        out=sS[:, :w],
                                        in_=ps[:, a : a + w],
                                        func=AF.Square,
                                        accum_out=SQL[:, r : r + 1],
                                    )
                            # t * xv for all g rows (xv broadcast over rows)
                            if g == 1:
                                nc.vector.tensor_mul(
                                    prodj[:, s * n : (s + 1) * n],
                                    ps[:, :n],
                                    xv_b[:, :w],
                                )
                            else:
                                nc.vector.tensor_mul(
                                    prodj[:, s * n : (s + 1) * n].rearrange(
                                        "p (g w) -> p g w", w=w
                                    ),
                                    ps[:, :n].rearrange("p (g w) -> p g w", w=w),
                                    xv_b[:, :w].unsqueeze(1).broadcast_to(
                                        [BCH, g, w]
                                    ),
                                )
                        nc.vector.tensor_reduce(
                            out=DOTL[:, j * 8 : (j + 1) * 8],
                            in_=prodj[:, : 8 * w].rearrange("p (r w) -> p r w", w=w),
                            axis=mybir.AxisListType.X,
                            op=ALU.add,
                        )
                        if j in BATCH_SQ:
                            nc.vector.tensor_reduce(
                                out=SQL[:, j * 8 : (j + 1) * 8],
                                in_=sqbj[:, : 8 * w].rearrange(
                                    "p (r w) -> p r w", w=w
                                ),
                                axis=mybir.AxisListType.X,
                                op=ALU.add,
                            )
                    st[bc] = dict(pex=pex, pdg=pdg, SQL=SQL, DOTL=DOTL)

                # ---- phase B: diag elementwise (all Exp — one table set) ----
                for bc in range(2):
                    s_ = st[bc]
                    pdg = s_["pdg"]
                    expd = scr.tile(
                        [BCH, NLOC * FIN], f32, name="expd", tag="expd", bufs=2
                    )
                    nc.scalar.activation(out=expd, in_=pdg, func=AF.Exp)
                    sq2 = scr.tile(
                        [BCH, NLOC * FIN], f32, name="sq2", tag="sq2", bufs=2
                    )
                    nc.scalar.activation(out=sq2, in_=pdg, func=AF.Exp, scale=2.0)
                    SQD = accs.tile([BCH, NLOC], f32, name="SQD", tag="SQD")
                    nc.vector.tensor_reduce(
                        out=SQD,
                        in_=sq2.rearrange("p (r f) -> p r f", f=FIN),
                        axis=mybir.AxisListType.X,
                        op=ALU.add,
                    )
                    prd = scr.tile(
                        [BCH, NLOC * FIN], f32, name="prd", tag="prd", bufs=2
                    )
                    nc.gpsimd.tensor_mul(prd, expd, xvd_sb[bc])
                    DOTD = accs.tile([BCH, NLOC], f32, name="DOTD", tag="DOTD")
                    nc.vector.tensor_reduce(
                        out=DOTD,
                        in_=prd.rearrange("p (r f) -> p r f", f=FIN),
                        axis=mybir.AxisListType.X,
                        op=ALU.add,
                    )
                    prl = scr.tile(
                        [BCH, NLOC * FIN], f32, name="prl", tag="prl", bufs=2
                    )
                    nc.gpsimd.tensor_mul(prl, expd, xle_sb[bc])
                    LDS = accs.tile([BCH, NLOC], f32, name="LDS", tag="LDS")
                    nc.vector.tensor_reduce(
                        out=LDS,
                        in_=prl.rearrange("p (r f) -> p r f", f=FIN),
                        axis=mybir.AxisListType.X,
                        op=ALU.add,
                    )
                    s_.update(SQD=SQD, DOTD=DOTD, LDS=LDS)

                # ---- phase C: assembly. ScalarE order: Ln*4, Copy*2, Exp*2 ----
                for bc in range(2):
                    s_ = st[bc]
                    sqn = accs.tile([BCH, NLOC], f32, name="sqn", tag="sqn")
                    nc.vector.tensor_add(sqn, s_["SQL"], s_["SQD"])
                    nc.vector.tensor_add(
                        sqn, sqn, s_["pex"][:, 2 * NLOC : 3 * NLOC]
                    )
                    dot = accs.tile([BCH, NLOC], f32, name="dot", tag="dot")
                    nc.vector.tensor_add(dot, s_["DOTL"], s_["DOTD"])
                    nc.vector.tensor_add(dot, dot, s_["pex"][:, 3 * NLOC :])
                    s_.update(sqn=sqn, dot=dot)
                for bc in range(2):
                    s_ = st[bc]
                    l1 = accs.tile([BCH, NLOC], f32, name="l1", tag="l1")
                    nc.scalar.activation(out=l1, in_=s_["sqn"], func=AF.Ln)
                    l2 = accs.tile([BCH, NLOC], f32, name="l2", tag="l2")
                    nc.scalar.activation(out=l2, in_=s_["LDS"], func=AF.Ln)
                    s_.update(l1=l1, l2=l2)
                for bc in range(2):
                    s_ = st[bc]
                    m1 = accs.tile([BCH, NLOC], f32, name="m1", tag="m1")
                    nc.scalar.mul(m1, s_["l1"], -0.5)
                    u = accs.tile([BCH, NLOC], f32, name="u", tag="u")
                    nc.vector.tensor_add(u, s_["pex"][:, :NLOC], m1)
                    s_.update(u=u)
                for bc in range(2):
                    s_ = st[bc]
                    sc = accs.tile([BCH, NLOC], f32, name="sc", tag="sc")
                    nc.scalar.activation(out=sc, in_=s_["u"], func=AF.Exp)
                    s_.update(sc=sc)
                for bc in range(2):
                    b0 = bc * BCH
                    s_ = st[bc]
                    yv = accs.tile([BCH, NLOC], f32, name="yv", tag="yv")
                    nc.vector.tensor_mul(yv, s_["dot"], s_["sc"])
                    yb = accs.tile([BCH, NLOC], f32, name="yb", tag="yb")
                    nc.vector.tensor_add(yb, yv, s_["pex"][:, NLOC : 2 * NLOC])
                    ld = accs.tile([BCH, NLOC], f32, name="ld", tag="ld")
                    nc.vector.tensor_add(ld, s_["u"], s_["l2"])
                    ob = accs.tile([BCH, NLOC, 2], f32, name="ob", tag="ob")
                    nc.gpsimd.tensor_copy(out=ob[:, :, 0], in_=yb)
                    nc.gpsimd.tensor_copy(out=ob[:, :, 1], in_=ld)
                    nc.sync.dma_start(out=d_out[b0 : b0 + BCH, :, :], in_=ob)

    nc.compile()
    return nc


def _host_prep(x, c, Wd, bd, Wa, ba, Wb, bb):
    """Build the 8 per-core input maps."""
    import ml_dtypes

    bf = ml_dtypes.bfloat16
    x = np.ascontiguousarray(x, dtype=np.float32)
    c = np.ascontiguousarray(c, dtype=np.float32)
    Wd5 = np.ascontiguousarray(Wd, dtype=np.float32).reshape(CDIM, NCH, FOUT, NCH, FIN)
    bd4 = np.ascontiguousarray(bd, dtype=np.float32).reshape(NCH, FOUT, NCH, FIN)
    Wa = np.ascontiguousarray(Wa, dtype=np.float32)
    Wb = np.ascontiguousarray(Wb, dtype=np.float32)
    ba = np.ascontiguousarray(ba, dtype=np.float32)
    bb = np.ascontiguousarray(bb, dtype=np.float32)

    cT = np.ascontiguousarray(c.T)
    xv = np.ascontiguousarray(x[:, :, 0])
    xl = np.ascontiguousarray(x[:, :, 1])
    xvT = np.ascontiguousarray(xv.T)

    in_maps = []
    for k in range(NCORES):
        chs = _channels(k)
        wdl = np.zeros((CDIM, TOTW), dtype=np.float32)
        wdd = np.empty((CDIM, NLOC * FIN), dtype=np.float32)
        bdd = np.empty((1, NLOC * FIN), dtype=np.float32)
        ew = np.zeros((CDIM, 3 * NLOC), dtype=np.float32)
        eb = np.zeros((1, 3 * NLOC), dtype=np.float32)
        bdm = np.zeros((I, NLOC), dtype=np.float32)
        xvd = np.empty((B, NLOC * FIN), dtype=np.float32)
        xled = np.empty((B, NLOC * FIN), dtype=np.float32)
        for j, ch in enumerate(chs):
            w = W_OF_J[j]
            for fo in range(FOUT):
                lo = BASE_J[j] + fo * w
                wdl[:, lo : lo + ch * FIN] = Wd5[:, ch, fo, :ch, :].reshape(CDIM, -1)
                r = j * FOUT + fo
                wdd[:, r * FIN : (r + 1) * FIN] = Wd5[:, ch, fo, ch, :]
                bdd[0, r * FIN : (r + 1) * FIN] = bd4[ch, fo, ch, :]
                bd_low = bd4[ch, fo, :ch, :].reshape(-1)  # true lower bd row
                bdm[: ch * FIN, r] = bd_low
                # sqn cross term 2*(c@g) and constant sum(bd^2)
                wl = Wd5[:, ch, fo, :ch, :].reshape(CDIM, -1)
                ew[:, 2 * NLOC + r] = 2.0 * (wl @ bd_low)
                eb[0, 2 * NLOC + r] = np.dot(bd_low, bd_low)
            rows = slice(ch * FOUT, (ch + 1) * FOUT)
            ew[:, j * FOUT : (j + 1) * FOUT] = Wa[:, rows]
            ew[:, NLOC + j * FOUT : NLOC + (j + 1) * FOUT] = Wb[:, rows]
            eb[0, j * FOUT : (j + 1) * FOUT] = ba[rows]
            eb[0, NLOC + j * FOUT : NLOC + (j + 1) * FOUT] = bb[rows]
        for r in range(NLOC):
            j = r // FOUT
            ch = chs[j]
            xvd[:, r * FIN : (r + 1) * FIN] = xv[:, ch * FIN : (ch + 1) * FIN]
            xled[:, r * FIN : (r + 1) * FIN] = xl[:, ch * FIN : (ch + 1) * FIN]
        np.exp(xled, out=xled)

        in_maps.append(
            {
                "ct": cT.astype(bf),
                "wdl": wdl.astype(bf),
                "wdd": wdd.astype(bf),
                "bdd": bdd.astype(bf),
                "ew": ew.astype(bf),
                "eb": eb.astype(bf),
                "xv": xv,
                "xvt": xvT.astype(bf),
                "bdm": bdm.astype(bf),
                "xvd": xvd,
                "xle": xled,
            }
        )
    return in_maps


def kernel(x, c, Wd, bd, Wa, ba, Wb, bb, _trace=False, _tmpdir=None):
    global _NC
    from concourse.bass_utils import run_bass_kernel_spmd

    if _NC is None:
        _NC = _build_nc()
    in_maps = _host_prep(x, c, Wd, bd, Wa, ba, Wb, bb)
    res = run_bass_kernel_spmd(
        _NC, in_maps, core_ids=list(range(NCORES)), trace=_trace, tmpdir=_tmpdir
    )

    out = np.empty((B, O, 2), dtype=np.float32)
    for k in range(NCORES):
        ok = res.results[k]["out"]
        for j, ch in enumerate(_channels(k)):
            out[:, ch * FOUT : (ch + 1) * FOUT, :] = ok[:, j * FOUT : (j + 1) * FOUT, :]
    if _trace:
        return out, res
    return out


# revision 19
# speedup vs baseline: 1.0141x; 1.0141x over previous
"""Trainium2 Bass kernel for nn_CLinear_6768868459230.

Context-conditioned block-autoregressive linear layer (MAF-style):
  wdir = c @ Wd + bd                      [B, O, I]
  w    = exp(wdir)*mask_diag + wdir*mask_lower
  sqn  = sum(w^2, axis=I)
  y    = (w / sqrt(sqn) * exp(wamp)) @ xv + bias
  logdet = logsumexp over diag block of (wdir - 0.5 log sqn + wamp + xl)

Sharding: tensor-parallel over the O=512 output rows (the 262144-wide Wd
matmul dominates). Each of the 8 cores owns 8 of the 64 channels, chosen
as pairs {k, 15-k, 16+k, 31-k, ...} so the block-triangular work (rows of
channel ch touch only ch*8 input columns) is identical on every core —
required anyway because all cores execute one shared program.

Only the strictly-lower + diagonal columns of Wd are shipped/loaded
(the rest are masked to zero by the reference), roughly halving traffic.
Per-row lower widths are zero-padded up to a per-slot maximum W(j)=64j+56
so the instruction stream is core-independent; zero padding is exact
(contributes 0 to both sums).

On-device per core (per 128-sample batch chunk):
  TensorE : wdir lower segments + diag block via cT.T @ Wd (float32r),
            with bd added as K=1 ones-outer-product accumulating matmuls
  ScalarE : per-row sum(t^2) via activation(Square, accum_out)
  VectorE : per-row sum(t * xv) via tensor_tensor_reduce
  diag    : batched 512-wide: exp(td), exp(2 td), products with xv and
            exp(xl), segmented tensor_reduce over fin=8
  logdet  = wamp - 0.5*ln(sqn) + ln(sum_f exp(td + xl))  (no max-trick
            needed: |td + xl| <~ 8 at these scales)
"""

import numpy as np

NCH, FIN, FOUT, CDIM, B = 64, 8, 8, 128, 256
I = NCH * FIN
O = NCH * FOUT
NCORES = 8
NLOC = 64  # output rows per core
BCH = 128  # batch chunk (SBUF partitions)

# per-slot padded lower width and rows-per-matmul grouping
W_OF_J = [64 * j + 56 for j in range(8)]
G_OF_J = [8, 4, 2, 2, 1, 1, 1, 1]  # rows per matmul so N = G*W <= 512
BASE_J = [8 * sum(W_OF_J[:j]) for j in range(8)]
TOTW = 8 * sum(W_OF_J)  # 17920


def _channels(k):
    return [k, 15 - k, 16 + k, 31 - k, 32 + k, 47 - k, 48 + k, 63 - k]


_NC = None
_REPEAT = 1  # bench knob: replicate compute body


def _build_nc():
    import concourse.bacc as bacc
    import concourse.tile as tile
    from concourse import mybir

    f32 = mybir.dt.float32
    f32r = mybir.dt.float32r
    bf16 = mybir.dt.bfloat16
    AF = mybir.ActivationFunctionType
    ALU = mybir.AluOpType

    nc = bacc.Bacc(None, target_bir_lowering=False)

    d_cT = nc.dram_tensor("ct", [CDIM, B], bf16, kind="ExternalInput")
    d_wdl = nc.dram_tensor("wdl", [CDIM, TOTW], bf16, kind="ExternalInput")
    d_wdd = nc.dram_tensor("wdd", [CDIM, NLOC * FIN], bf16, kind="ExternalInput")
    d_bdd = nc.dram_tensor("bdd", [1, NLOC * FIN], bf16, kind="ExternalInput")
    d_ew = nc.dram_tensor("ew", [CDIM, 3 * NLOC], bf16, kind="ExternalInput")
    d_eb = nc.dram_tensor("eb", [1, 3 * NLOC], bf16, kind="ExternalInput")
    d_xvT = nc.dram_tensor("xvt", [I, B], bf16, kind="ExternalInput")
    d_bdm = nc.dram_tensor("bdm", [I, NLOC], bf16, kind="ExternalInput")
    d_xv = nc.dram_tensor("xv", [B, I], f32, kind="ExternalInput")
    d_xvd = nc.dram_tensor("xvd", [B, NLOC * FIN], f32, kind="ExternalInput")
    d_xle = nc.dram_tensor("xle", [B, NLOC * FIN], f32, kind="ExternalInput")
    d_out = nc.dram_tensor("out", [B, NLOC, 2], f32, kind="ExternalOutput")

    with tile.TileContext(nc) as tc:
        with (
            tc.tile_pool(name="consts", bufs=1) as consts,
            tc.tile_pool(name="scr", bufs=3) as scr,
            tc.tile_pool(name="accs", bufs=2) as accs,
            tc.tile_pool(name="segp", bufs=6, space="PSUM") as segp,
            tc.tile_pool(name="miscp", bufs=1, space="PSUM") as miscp,
            tc.tile_pool(name="extp", bufs=1, space="PSUM") as extp,
        ):
            # ---- constants / weights ----
            # sync queue: ct + xv first (gate the matmuls / dot products),
            # then even weight slots. scalar queue: small matmul operands,
            # then odd weight slots. The two HW DGEs stream in parallel.
            ct_sb = consts.tile([CDIM, B], bf16)
            nc.sync.dma_start(out=ct_sb, in_=d_cT[:, :])
            ones_sb = consts.tile([1, BCH], bf16)
            nc.vector.memset(ones_sb, 1.0)
            xv_sb, xvd_sb, xle_sb = [], [], []
            for bc in range(2):
                b0 = bc * BCH
                t = consts.tile([BCH, I], f32, name=f"xv{bc}", tag=f"xv{bc}")
                nc.sync.dma_start(out=t, in_=d_xv[b0 : b0 + BCH, :])
                xv_sb.append(t)
            wdd_sb = consts.tile([CDIM, NLOC * FIN], bf16)
            nc.scalar.dma_start(out=wdd_sb, in_=d_wdd[:, :])
            bdd_sb = consts.tile([1, NLOC * FIN], bf16)
            nc.scalar.dma_start(out=bdd_sb, in_=d_bdd[:, :])
            ew_sb = consts.tile([CDIM, 3 * NLOC], bf16)
            nc.scalar.dma_start(out=ew_sb, in_=d_ew[:, :])
            eb_sb = consts.tile([1, 3 * NLOC], bf16)
            nc.scalar.dma_start(out=eb_sb, in_=d_eb[:, :])
            xvt_sb = consts.tile([CDIM, 4, B], bf16)
            for kc in range(4):
                nc.scalar.dma_start(
                    out=xvt_sb[:, kc, :], in_=d_xvT[kc * 128 : (kc + 1) * 128, :]
                )
            bdm_sb = consts.tile([CDIM, 4, NLOC], bf16)
            for kc in range(4):
                nc.scalar.dma_start(
                    out=bdm_sb[:, kc, :], in_=d_bdm[kc * 128 : (kc + 1) * 128, :]
                )
            wdl_sb = [None] * 8
            for j in (0, 1, 2, 3, 4, 5, 6, 7):
                w = W_OF_J[j]
                t = consts.tile([CDIM, 8 * w], bf16, name=f"wdl{j}", tag=f"wdl{j}")
                eng = nc.sync if j % 2 == 0 else nc.scalar
                eng.dma_start(out=t, in_=d_wdl[:, BASE_J[j] : BASE_J[j] + 8 * w])
                wdl_sb[j] = t
            for bc in range(2):
                b0 = bc * BCH
                t = consts.tile([BCH, NLOC * FIN], f32, name=f"xvd{bc}", tag=f"xvd{bc}")
                nc.sync.dma_start(out=t, in_=d_xvd[b0 : b0 + BCH, :])
                xvd_sb.append(t)
                t = consts.tile([BCH, NLOC * FIN], f32, name=f"xle{bc}", tag=f"xle{bc}")
                nc.sync.dma_start(out=t, in_=d_xle[b0 : b0 + BCH, :])
                xle_sb.append(t)

            for _rep in range(_REPEAT):
                BATCH_SQ = (0, 1)  # slots whose squares are segment-batched
                st = {}
                # ---- phase A: matmuls + squares + dot products ----
                for bc in range(2):
                    b0 = bc * BCH
                    lhs = ct_sb[:, b0 : b0 + BCH]
                    xv_b = xv_sb[bc]

                    # extras: wamp | bias | 2*c@g | dotbd
                    pex = extp.tile([BCH, 4 * NLOC], f32, name="pex", tag="pex")
                    nc.tensor.matmul(
                        pex[:, : 3 * NLOC], lhs, ew_sb, start=True, stop=False
                    )
                    nc.tensor.matmul(
                        pex[:, : 3 * NLOC], ones_sb, eb_sb, start=False, stop=True
                    )
                    for kc in range(4):
                        nc.tensor.matmul(
                            pex[:, 3 * NLOC :],
                            xvt_sb[:, kc, b0 : b0 + BCH],
                            bdm_sb[:, kc, :],
                            start=(kc == 0),
                            stop=(kc == 3),
                        )

                    # diag block matmul (elementwise work deferred to phase B)
                    pdg = miscp.tile([BCH, NLOC * FIN], f32, name="pdg", tag="pdg")
                    nc.tensor.matmul(pdg, lhs, wdd_sb, start=True, stop=False)
                    nc.tensor.matmul(pdg, ones_sb, bdd_sb, start=False, stop=True)

                    SQL = accs.tile([BCH, NLOC], f32, name="SQL", tag="SQL")
                    DOTL = accs.tile([BCH, NLOC], f32, name="DOTL", tag="DOTL")
                    sq_pend = []
                    for j in range(8):
                        w, g = W_OF_J[j], G_OF_J[j]
                        prodj = scr.tile(
                            [BCH, 8 * 504], f32, name="prodj", tag="prodj", bufs=2
                        )
                        sqbj = None
                        if j in BATCH_SQ:
                            sqbj = scr.tile(
                                [BCH, 8 * 120], f32, name="sqbj", tag="sqbj", bufs=2
                            )
                        for s in range(8 // g):
                            r0 = j * 8 + s * g
                            n = g * w
                            ps = segp.tile([BCH, 512], f32, name="ps", tag="ps")
                            nc.tensor.matmul(
                                ps[:, :n],
                                lhs,
                                wdl_sb[j][:, s * n : (s + 1) * n],
                                start=True,
                                stop=True,
                            )
                            if j in BATCH_SQ:
                                nc.scalar.activation(
                                    out=sqbj[:, s * n : (s + 1) * n],
                                    in_=ps[:, :n],
                                    func=AF.Square,
                                )
                            else:
                                for q in range(g):
                                    r = r0 + q
                                    a = q * w
                                    sS = scr.tile(
                                        [BCH, 504], f32, name="sS", tag="sS"
                                    )
                                    nc.scalar.activation(
                                        out=sS[:, :w],
                                        in_=ps[:, a : a + w],
                                        func=AF.Square,
                                        accum_out=SQL[:, r : r + 1],
                                    )
                            # t * xv for all g rows (xv broadcast over rows)
                            if g == 1:
                                nc.vector.tensor_mul(
                                    prodj[:, s * n : (s + 1) * n],
                                    ps[:, :n],
                                    xv_b[:, :w],
                                )
                            else:
                                nc.vector.tensor_mul(
                                    prodj[:, s * n : (s + 1) * n].rearrange(
                                        "p (g w) -> p g w", w=w
                                    ),
                                    ps[:, :n].rearrange("p (g w) -> p g w", w=w),
                                    xv_b[:, :w].unsqueeze(1).broadcast_to(
                                        [BCH, g, w]
                                    ),
                                )
                        nc.vector.tensor_reduce(
                            out=DOTL[:, j * 8 : (j + 1) * 8],
                            in_=prodj[:, : 8 * w].rearrange("p (r w) -> p r w", w=w),
                            axis=mybir.AxisListType.X,
                            op=ALU.add,
                        )
                        if j in BATCH_SQ:
                            sq_pend.append((j, w, sqbj))
                    for j, w, sqbj in sq_pend:
                        nc.vector.tensor_reduce(
                            out=SQL[:, j * 8 : (j + 1) * 8],
                            in_=sqbj[:, : 8 * w].rearrange("p (r w) -> p r w", w=w),
                            axis=mybir.AxisListType.X,
                            op=ALU.add,
                        )
                    st[bc] = dict(pex=pex, pdg=pdg, SQL=SQL, DOTL=DOTL)

                # ---- phase B: diag elementwise (all Exp — one table set) ----
                for bc in range(2):
                    s_ = st[bc]
                    pdg = s_["pdg"]
                    expd = scr.tile(
                        [BCH, NLOC * FIN], f32, name="expd", tag="expd", bufs=2
                    )
                    nc.scalar.activation(out=expd, in_=pdg, func=AF.Exp)
                    sq2 = scr.tile(
                        [BCH, NLOC * FIN], f32, name="sq2", tag="sq2", bufs=2
                    )
                    nc.scalar.activation(out=sq2, in_=pdg, func=AF.Exp, scale=2.0)
                    SQD = accs.tile([BCH, NLOC], f32, name="SQD", tag="SQD")
                    nc.vector.tensor_reduce(
                        out=SQD,
                        in_=sq2.rearrange("p (r f) -> p r f", f=FIN),
                        axis=mybir.AxisListType.X,
                        op=ALU.add,
                    )
                    prd = scr.tile(
                        [BCH, NLOC * FIN], f32, name="prd", tag="prd", bufs=2
                    )
                    nc.gpsimd.tensor_mul(prd, expd, xvd_sb[bc])
                    DOTD = accs.tile([BCH, NLOC], f32, name="DOTD", tag="DOTD")
                    nc.vector.tensor_reduce(
                        out=DOTD,
                        in_=prd.rearrange("p (r f) -> p r f", f=FIN),
                        axis=mybir.AxisListType.X,
                        op=ALU.add,
                    )
                    prl = scr.tile(
                        [BCH, NLOC * FIN], f32, name="prl", tag="prl", bufs=2
                    )
                    nc.gpsimd.tensor_mul(prl, expd, xle_sb[bc])
                    LDS = accs.tile([BCH, NLOC], f32, name="LDS", tag="LDS")
                    nc.vector.tensor_reduce(
                        out=LDS,
                        in_=prl.rearrange("p (r f) -> p r f", f=FIN),
                        axis=mybir.AxisListType.X,
                        op=ALU.add,
                    )
                    s_.update(SQD=SQD, DOTD=DOTD, LDS=LDS)

                # ---- phase C: assembly. ScalarE order: Ln*4, Copy*2, Exp*2 ----
                for bc in range(2):
                    s_ = st[bc]
                    sqn = accs.tile([BCH, NLOC], f32, name="sqn", tag="sqn")
                    nc.vector.tensor_add(sqn, s_["SQL"], s_["SQD"])
                    nc.vector.tensor_add(
                        sqn, sqn, s_["pex"][:, 2 * NLOC : 3 * NLOC]
                    )
                    dot = accs.tile([BCH, NLOC], f32, name="dot", tag="dot")
                    nc.vector.tensor_add(dot, s_["DOTL"], s_["DOTD"])
                    nc.vector.tensor_add(dot, dot, s_["pex"][:, 3 * NLOC :])
                    s_.update(sqn=sqn, dot=dot)
                for bc in range(2):
                    s_ = st[bc]
                    l1 = accs.tile([BCH, NLOC], f32, name="l1", tag="l1")
                    nc.scalar.activation(out=l1, in_=s_["sqn"], func=AF.Ln)
                    l2 = accs.tile([BCH, NLOC], f32, name="l2", tag="l2")
                    nc.scalar.activation(out=l2, in_=s_["LDS"], func=AF.Ln)
                    s_.update(l1=l1, l2=l2)
                for bc in range(2):
                    s_ = st[bc]
                    m1 = accs.tile([BCH, NLOC], f32, name="m1", tag="m1")
                    nc.scalar.mul(m1, s_["l1"], -0.5)
                    u = accs.tile([BCH, NLOC], f32, name="u", tag="u")
                    nc.vector.tensor_add(u, s_["pex"][:, :NLOC], m1)
                    s_.update(u=u)
                for bc in range(2):
                    s_ = st[bc]
                    sc = accs.tile([BCH, NLOC], f32, name="sc", tag="sc")
                    nc.scalar.activation(out=sc, in_=s_["u"], func=AF.Exp)
                    s_.update(sc=sc)
                for bc in range(2):
                    b0 = bc * BCH
                    s_ = st[bc]
                    yv = accs.tile([BCH, NLOC], f32, name="yv", tag="yv")
                    nc.vector.tensor_mul(yv, s_["dot"], s_["sc"])
                    yb = accs.tile([BCH, NLOC], f32, name="yb", tag="yb")
                    nc.vector.tensor_add(yb, yv, s_["pex"][:, NLOC : 2 * NLOC])
                    ld = accs.tile([BCH, NLOC], f32, name="ld", tag="ld")
                    nc.vector.tensor_add(ld, s_["u"], s_["l2"])
                    ob = accs.tile([BCH, NLOC, 2], f32, name="ob", tag="ob")
                    nc.gpsimd.tensor_copy(out=ob[:, :, 0], in_=yb)
                    nc.gpsimd.tensor_copy(out=ob[:, :, 1], in_=ld)
                    nc.sync.dma_start(out=d_out[b0 : b0 + BCH, :, :], in_=ob)

    nc.compile()
    return nc


def _host_prep(x, c, Wd, bd, Wa, ba, Wb, bb):
    """Build the 8 per-core input maps."""
    import ml_dtypes

    bf = ml_dtypes.bfloat16
    x = np.ascontiguousarray(x, dtype=np.float32)
    c = np.ascontiguousarray(c, dtype=np.float32)
    Wd5 = np.ascontiguousarray(Wd, dtype=np.float32).reshape(CDIM, NCH, FOUT, NCH, FIN)
    bd4 = np.ascontiguousarray(bd, dtype=np.float32).reshape(NCH, FOUT, NCH, FIN)
    Wa = np.ascontiguousarray(Wa, dtype=np.float32)
    Wb = np.ascontiguousarray(Wb, dtype=np.float32)
    ba = np.ascontiguousarray(ba, dtype=np.float32)
    bb = np.ascontiguousarray(bb, dtype=np.float32)

    cT = np.ascontiguousarray(c.T)
    xv = np.ascontiguousarray(x[:, :, 0])
    xl = np.ascontiguousarray(x[:, :, 1])
    xvT = np.ascontiguousarray(xv.T)

    in_maps = []
    for k in range(NCORES):
        chs = _channels(k)
        wdl = np.zeros((CDIM, TOTW), dtype=np.float32)
        wdd = np.empty((CDIM, NLOC * FIN), dtype=np.float32)
        bdd = np.empty((1, NLOC * FIN), dtype=np.float32)
        ew = np.zeros((CDIM, 3 * NLOC), dtype=np.float32)
        eb = np.zeros((1, 3 * NLOC), dtype=np.float32)
        bdm = np.zeros((I, NLOC), dtype=np.float32)
        xvd = np.empty((B, NLOC * FIN), dtype=np.float32)
        xled = np.empty((B, NLOC * FIN), dtype=np.float32)
        for j, ch in enumerate(chs):
            w = W_OF_J[j]
            for fo in range(FOUT):
                lo = BASE_J[j] + fo * w
                wdl[:, lo : lo + ch * FIN] = Wd5[:, ch, fo, :ch, :].reshape(CDIM, -1)
                r = j * FOUT + fo
                wdd[:, r * FIN : (r + 1) * FIN] = Wd5[:, ch, fo, ch, :]
                bdd[0, r * FIN : (r + 1) * FIN] = bd4[ch, fo, ch, :]
                bd_low = bd4[ch, fo, :ch, :].reshape(-1)  # true lower bd row
                bdm[: ch * FIN, r] = bd_low
                # sqn cross term 2*(c@g) and constant sum(bd^2)
                wl = Wd5[:, ch, fo, :ch, :].reshape(CDIM, -1)
                ew[:, 2 * NLOC + r] = 2.0 * (wl @ bd_low)
                eb[0, 2 * NLOC + r] = np.dot(bd_low, bd_low)
            rows = slice(ch * FOUT, (ch + 1) * FOUT)
            ew[:, j * FOUT : (j + 1) * FOUT] = Wa[:, rows]
            ew[:, NLOC + j * FOUT : NLOC + (j + 1) * FOUT] = Wb[:, rows]
            eb[0, j * FOUT : (j + 1) * FOUT] = ba[rows]
            eb[0, NLOC + j * FOUT : NLOC + (j + 1) * FOUT] = bb[rows]
        for r in range(NLOC):
            j = r // FOUT
            ch = chs[j]
            xvd[:, r * FIN : (r + 1) * FIN] = xv[:, ch * FIN : (ch + 1) * FIN]
            xled[:, r * FIN : (r + 1) * FIN] = xl[:, ch * FIN : (ch + 1) * FIN]
        np.exp(xled, out=xled)

        in_maps.append(
            {
                "ct": cT.astype(bf),
                "wdl": wdl.astype(bf),
                "wdd": wdd.astype(bf),
                "bdd": bdd.astype(bf),
                "ew": ew.astype(bf),
                "eb": eb.astype(bf),
                "xv": xv,
                "xvt": xvT.astype(bf),
                "bdm": bdm.astype(bf),
                "xvd": xvd,
                "xle": xled,
            }
        )
    return in_maps


def kernel(x, c, Wd, bd, Wa, ba, Wb, bb, _trace=False, _tmpdir=None):
    global _NC
    from concourse.bass_utils import run_bass_kernel_spmd

    if _NC is None:
        _NC = _build_nc()
    in_maps = _host_prep(x, c, Wd, bd, Wa, ba, Wb, bb)
    res = run_bass_kernel_spmd(
        _NC, in_maps, core_ids=list(range(NCORES)), trace=_trace, tmpdir=_tmpdir
    )

    out = np.empty((B, O, 2), dtype=np.float32)
    for k in range(NCORES):
        ok = res.results[k]["out"]
        for j, ch in enumerate(_channels(k)):
            out[:, ch * FOUT : (ch + 1) * FOUT, :] = ok[:, j * FOUT : (j + 1) * FOUT, :]
    if _trace:
        return out, res
    return out
